# Initial kernel scaffold
#
"""Trainium2 bass kernel for nn_CM_41162966565199 (dense_cnn, dynamic filter).

Computation (per batch sample):
  filt = Conv2d(C=64 -> 9C=576, 3x3, pad=1)(gt) + bias          # dynamic filters
  out[c,h,w] = sum_j filt[c*9+j, h, w] * patches_j(gr)[c, h, w] # 3x3 dyn. filter

Strategy: pure data parallel, one sample per NeuronCore (N=8, 8 cores).

Per core:
- Conv as shift-based matmuls in float32r (full PE rate at N>=256):
  contraction (in_channel i, tap p) tiled into 5 K=128 chunks by pairing taps
  whose flat-offset delta is +1 (or +132), realized by stacking two shifted
  copies of gt on SBUF partitions 0-63 / 64-127. Output channels (c, j) tiled
  into 5 M-tiles of two j-groups each. All matmuls K=128, M=128, N=512.
- Dynamic-filter stage on DVE: scalar_tensor_tensor fuses (psum + bias) * gr
  reading PSUM directly; products accumulated pairwise; the upper/lower
  partition halves hold disjoint partial sums, folded on the host.
- Spatial flattening uses a 2-ring padded 132x132 grid so every 3x3 tap is a
  pure flat offset; host pre-pads (zeros for conv, replicate ring for
  patches) and crops/folds the 130x132 output grid back to 128x128.
"""

import numpy as np

import concourse.bass as bass
import concourse.mybir as mybir
import concourse.tile as tile
from concourse import bacc
from concourse.bass_utils import run_bass_kernel_spmd
from concourse.vector_clock import ScopedClock

# ---------------------------------------------------------------- constants
N, C, H, W, KS = 8, 64, 128, 128, 3
W2 = W + 4                      # 132: 2-ring padded row width
NROW = H + 4                    # 132 padded rows
NOUT = (H + 2) * W2             # 17160: output grid (130 rows x 132 cols)
NTILE = 512
NT = -(-NOUT // NTILE)          # 34 spatial tiles
OUT_LEN = NT * NTILE            # 17408
FLAT_SRC = 18944                # padded flat source length (covers max reads)
NB = 4                          # N-tiles per block (weight reuse)
WIN = NB * NTILE + 272          # 2320: rhs window width per block

F32 = mybir.dt.float32
F32R = mybir.dt.float32r
F16 = mybir.dt.float16
ADD = mybir.AluOpType.add
MULT = mybir.AluOpType.mult

# 5 K-chunks over the 9 conv taps p=(kh,kw); flat offset d_p = kh*132+kw.
# Pairs (p_a, p_b): upper/lower SBUF partition halves. Chunks 0-2 pair
# (kh,0)+(kh,1) (delta=1, gtAB buffer), chunk 3 pairs (0,2)+(1,2)
# (delta=132, gtAC buffer), chunk 4 is the lone (2,2) with zeroed lower
# weights.
CHUNKS = [((0, 0), (0, 1)), ((1, 0), (1, 1)), ((2, 0), (2, 1)),
          ((0, 2), (1, 2)), ((2, 2), None)]
# 5 M-tiles: which two j-groups (of the 9 output filter taps) share a PSUM
# tile's upper/lower 64 partitions.
MTILES = CHUNKS


# ------------------------------------------------- TileContext drain patch
# This walrus build rejects >2 sync-wait commands on one CTRL instruction;
# the stock TileContext tail hangs every pending sem wait on a single SP
# Drain. Split them across single-wait SP NOPs (program order on SP still
# places them before the barrier + sem reset).
def _drain_and_barrier_split(self, tick_clock, wait_clock):
    nc = self.nc
    drain_inst = nc.sync.drain()
    wait_clock.add_sem_waits(
        drain_inst.ins, ScopedClock({None: tick_clock.global_clock})
    )
    si = drain_inst.ins.sync_info
    if si is not None and len(si.on_wait) > 1:
        waits = list(si.on_wait)
        drain_inst.ins.sync_info = mybir.SyncInfo(on_wait=[waits[0]], on_update=[])
        for w in waits[1:]:
            nop = nc.sync.nop()
            nop.ins.sync_info = mybir.SyncInfo(on_wait=[w], on_update=[])
    nc.all_engine_barrier()
    assert self.sems is not None
    popped = nc._tile_sem_poison_stack.pop()
    assert popped is self._sem_poison
    nc.clear_and_free_semaphores(list(self.sems.allocated().values()))
    nc.all_engine_barrier()


tile.TileContext._drain_and_barrier = _drain_and_barrier_split


# ------------------------------------------------------------- host prep
def _prep_gt(gt):
    """[C,H,W] -> [C, FLAT_SRC] flat 132x132 grid, 2-ring zero pad."""
    buf = np.zeros((C, FLAT_SRC), np.float32)
    pad = np.zeros((C, NROW, W2), np.float32)
    pad[:, 2:2 + H, 2:2 + W] = gt
    buf[:, :NROW * W2] = pad.reshape(C, -1)
    return buf


def _prep_gr(gr):
    """[C,H,W] -> flat 132x132 grid; inner 130x130 = replicate-padded gr."""
    rp = np.pad(gr, ((0, 0), (1, 1), (1, 1)), mode="edge")
    pad = np.zeros((C, NROW, W2), np.float32)
    pad[:, 1:3 + H, 1:3 + W] = rp
    buf = np.zeros((C, FLAT_SRC), np.float32)
    buf[:, :NROW * W2] = pad.reshape(C, -1)
    return buf


def _jidx(j):
    return j[0] * 3 + j[1]


def _prep_w(Wc):
    """[576,64,3,3] -> [25,128,128] lhsT blocks [(m_tile,chunk), K, M]."""
    out = np.zeros((5, 5, 128, 128), np.float32)
    cc = np.arange(C)
    for m, (j0, j1) in enumerate(MTILES):
        for c, (pa, pb) in enumerate(CHUNKS):
            for hk, p in ((0, pa), (1, pb)):
                if p is None:
                    continue
                kh, kw = p
                for hm, j in ((0, j0), (1, j1)):
                    if j is None:
                        continue
                    blk = Wc[cc * 9 + _jidx(j), :, kh, kw]  # [c_out, i]
                    out[m, c, 64 * hk:64 * hk + 64, 64 * hm:64 * hm + 64] = blk.T
    # partition-major [128, 25*128] so the device load is one plain 2D DMA
    return np.ascontiguousarray(
        out.reshape(25, 128, 128).transpose(1, 0, 2).reshape(128, 25 * 128)
    )


def _prep_b(bc):
    """[576] -> [128,5] per-M-tile per-partition bias (partition-major)."""
    out = np.zeros((5, 128), np.float32)
    cc = np.arange(C)
    for m, (j0, j1) in enumerate(MTILES):
        for hm, j in ((0, j0), (1, j1)):
            if j is None:
                continue
            out[m, 64 * hm:64 * hm + 64] = bc[cc * 9 + _jidx(j)]
    return np.ascontiguousarray(out.T)


# --------------------------------------------------------- bass program
def _build():
    # Bacc (not plain Bass): its finalize() -> compile() legalizes the
    # multi-wait instructions Tile emits (move_matmul_waits_to_ldweights,
    # generate_event_semaphores) which this walrus build otherwise rejects
    # with "Too many sync wait commands".
    nc = bacc.Bacc(None, target_bir_lowering=False)
    gt_src = nc.dram_tensor("gt_src", [C, FLAT_SRC], F32R, kind="ExternalInput")
    gr_src = nc.dram_tensor("gr_src", [C, FLAT_SRC], F32, kind="ExternalInput")
    w_src = nc.dram_tensor("w_src", [128, 25 * 128], F32R, kind="ExternalInput")
    b_src = nc.dram_tensor("b_src", [128, 5], F32, kind="ExternalInput")
    o_dst = nc.dram_tensor("o_dst", [128, OUT_LEN], F16, kind="ExternalOutput")

    sizes = [4] * 8 + [2]
    assert sum(sizes) == NT
    blocks = []
    t0 = 0
    for nb in sizes:
        blocks.append((t0, nb))
        t0 += nb

    # per-M-tile gr source for the product stage: (buffer, flat offset);
    # m3 reads the delta-132 pair buffer full-width, m4 is a half op.
    with tile.TileContext(nc) as tc:
        with (
            tc.tile_pool(name="wpool", bufs=1) as wpool,
            tc.tile_pool(name="winpool", bufs=2) as winpool,
            tc.tile_pool(name="pspool", bufs=4, space="PSUM") as pspool,
            tc.tile_pool(name="prodpool", bufs=12) as prodpool,
            tc.tile_pool(name="accpool", bufs=6) as accpool,
        ):
            # Weights land as partition-major 2D DMAs (a 3-dim AP DMA costs
            # ~11us to issue on SP; per-chunk loads cost ~0.6us each). One
            # DMA per M-tile so the first matmuls only wait for 320KB, not
            # the whole 1.6MB.
            wsb = wpool.tile([128, 25 * 128], F32R, name="wsb", tag="wsb")
            bias_sb = wpool.tile([128, 5], F32, name="bias_sb", tag="bias")

            def load_weights_m(m):
                nc.sync.dma_start(
                    out=wsb[:, m * 640:(m + 1) * 640],
                    in_=w_src[:, m * 640:(m + 1) * 640],
                )

            def stt(out_ap, ps_ap, b_ap, gr_ap):
                nc.vector.scalar_tensor_tensor(
                    out_ap, ps_ap, b_ap, gr_ap, op0=ADD, op1=MULT
                )

            def win_load(pool, name, src, base, pair_step, dtype, wneed):
                """Partitions 0-63 <- src[base+q], 64-127 <-
                src[base+pair_step+q], as two 2D DMAs of just the columns
                this block touches."""
                t = pool.tile([128, WIN], dtype, name=name, tag=name)
                nc.sync.dma_start(out=t[0:64, 0:wneed],
                                  in_=src[:, base:base + wneed])
                nc.sync.dma_start(
                    out=t[64:128, 0:wneed],
                    in_=src[:, base + pair_step:base + pair_step + wneed],
                )
                return t

            for bi, (t0, nb) in enumerate(blocks):
                T = t0 * NTILE
                wneed = nb * NTILE + 272
                gtab = win_load(winpool, "gtab", gt_src, T, 1, F32R, wneed)
                gtac = win_load(winpool, "gtac", gt_src, T + 2, 132, F32R, wneed)
                if bi == 0:
                    # first M-tile's weights right after the first gt windows
                    # so the first matmuls' inputs win the head DMA race;
                    # the rest follow the gr windows
                    load_weights_m(0)
                grab = win_load(winpool, "grab", gr_src, T, 1, F32, wneed)
                grac = win_load(winpool, "grac", gr_src, T + 2, 132, F32, wneed)
                if bi == 0:
                    for m in range(1, 5):
                        load_weights_m(m)
                    nc.sync.dma_start(out=bias_sb[:, :], in_=b_src[:, :])

                # Conv matmuls per M-tile (weights reused across the block's
                # N-tiles to amortize LDWEIGHTS). PSUM tiles span TWO banks
                # (two adjacent N-tiles) so the product/add stage runs
                # 1024-wide DVE ops — halves per-op overhead. Products are
                # written fp16 so the add tree hits the DVE 2x_1P mode.
                npair = nb // 2
                prods = [[None] * 5 for _ in range(npair)]
                for m in range(5):
                    pst = [
                        pspool.tile([128, 2 * NTILE], F32, name=f"ps{m}_{p}",
                                    tag="ps")
                        for p in range(npair)
                    ]
                    for c in range(5):
                        k = m * 5 + c
                        lhsT = wsb[:, k * 128:(k + 1) * 128]
                        for tb in range(nb):
                            q = tb * NTILE
                            if c < 3:
                                rhs = gtab[:, q + c * W2: q + c * W2 + NTILE]
                            elif c == 3:
                                rhs = gtac[:, q: q + NTILE]
                            else:
                                rhs = gtab[:, q + 266: q + 266 + NTILE]
                            out_ps = pst[tb // 2][:, (tb % 2) * NTILE:
                                                  (tb % 2 + 1) * NTILE]
                            nc.tensor.matmul(
                                out_ps, lhsT, rhs,
                                start=(c == 0), stop=(c == 4),
                            )
                    for p in range(npair):
                        q = 2 * p * NTILE
                        Wd = 2 * NTILE
                        pr = prodpool.tile(
                            [128, Wd], F16, name=f"m{m}", tag="prod"
                        )
                        prods[p][m] = pr
                        if m < 3:
                            stt(pr[:, :], pst[p][:, :], bias_sb[:, m:m + 1],
                                grab[:, q + m * W2: q + m * W2 + Wd])
                        elif m == 3:
                            stt(pr[:, :], pst[p][:, :], bias_sb[:, 3:4],
                                grac[:, q: q + Wd])
                        else:
                            stt(pr[0:64], pst[p][0:64], bias_sb[0:64, 4:5],
                                grab[0:64, q + 266: q + 266 + Wd])

                # fp16 pairwise sum tree on DVE (2x_1P mode)
                for p in range(npair):
                    t = t0 + 2 * p
                    Wd = 2 * NTILE
                    m1, m2, m3, m4, m5 = prods[p]
                    a1 = accpool.tile([128, Wd], F16, name="a1", tag="acc")
                    nc.vector.tensor_tensor(a1[:, :], m1[:, :], m2[:, :], op=ADD)
                    a2 = accpool.tile([128, Wd], F16, name="a2", tag="acc")
                    nc.vector.tensor_tensor(a2[:, :], m3[:, :], m4[:, :], op=ADD)
                    a3 = accpool.tile([128, Wd], F16, name="a3", tag="acc")
                    nc.vector.tensor_tensor(a3[:, :], a1[:, :], a2[:, :], op=ADD)
                    nc.vector.tensor_tensor(a3[0:64], a3[0:64], m5[0:64], op=ADD)
                    # out-DMA from the idle ACT queue: on the in-order SP
                    # queue it would park behind the block's last DVE adds
                    # and delay the next block's window loads (~5us PE stall
                    # per block boundary)
                    nc.scalar.dma_start(
                        out=o_dst[:, t * NTILE: t * NTILE + Wd], in_=a3[:, :]
                    )
    nc.finalize()
    return nc


_NC = None


def _get_nc():
    global _NC
    if _NC is None:
        _NC = _build()
    return _NC


_RUN_KW = {}  # test harness can inject trace=True etc.
_LAST_RESULT = None


def kernel(gr, gt, Wc, bc):
    global _LAST_RESULT
    gr = np.ascontiguousarray(np.asarray(gr, dtype=np.float32))
    gt = np.ascontiguousarray(np.asarray(gt, dtype=np.float32))
    Wc = np.asarray(Wc, dtype=np.float32)
    bc = np.asarray(bc, dtype=np.float32)

    wb = _prep_w(Wc)
    bb = _prep_b(bc)
    in_maps = [
        {
            "gt_src": _prep_gt(gt[n]),
            "gr_src": _prep_gr(gr[n]),
            "w_src": wb,
            "b_src": bb,
        }
        for n in range(N)
    ]
    res = run_bass_kernel_spmd(
        _get_nc(), in_maps, core_ids=list(range(N)), **_RUN_KW
    )
    _LAST_RESULT = res

    hh = np.arange(H)
    cols = ((hh + 1) * W2)[:, None] + (np.arange(W) + 1)[None, :]
    outs = []
    for n in range(N):
        O = res.results[n]["o_dst"].astype(np.float32)
        flat = O[:64] + O[64:]
        outs.append(flat[:, cols])
    return np.stack(outs).astype(np.float32)



# revision 1
# speedup vs baseline: 1.0076x; 1.0076x over previous
"""Trainium2 bass kernel for nn_CM_41162966565199 (dense_cnn, dynamic filter).

Computation (per batch sample):
  filt = Conv2d(C=64 -> 9C=576, 3x3, pad=1)(gt) + bias          # dynamic filters
  out[c,h,w] = sum_j filt[c*9+j, h, w] * patches_j(gr)[c, h, w] # 3x3 dyn. filter

Strategy: pure data parallel, one sample per NeuronCore (N=8, 8 cores).

Per core:
- Conv as shift-based matmuls in float32r (full PE rate at N>=256):
  contraction (in_channel i, tap p) tiled into 5 K=128 chunks by pairing taps
  whose flat-offset delta is +1 (or +132), realized by stacking two shifted
  copies of gt on SBUF partitions 0-63 / 64-127. Output channels (c, j) tiled
  into 5 M-tiles of two j-groups each. All matmuls K=128, M=128, N=512.
- Dynamic-filter stage on DVE: scalar_tensor_tensor fuses (psum + bias) * gr
  reading PSUM directly; products accumulated pairwise; the upper/lower
  partition halves hold disjoint partial sums, folded on the host.
- Spatial flattening uses a 2-ring padded 132x132 grid so every 3x3 tap is a
  pure flat offset; host pre-pads (zeros for conv, replicate ring for
  patches) and crops/folds the 130x132 output grid back to 128x128.
"""

import numpy as np

import concourse.bass as bass
import concourse.mybir as mybir
import concourse.tile as tile
from concourse import bacc
from concourse.bass_utils import run_bass_kernel_spmd
from concourse.vector_clock import ScopedClock

# ---------------------------------------------------------------- constants
N, C, H, W, KS = 8, 64, 128, 128, 3
W2 = W + 4                      # 132: 2-ring padded row width
NROW = H + 4                    # 132 padded rows
NOUT = (H + 2) * W2             # 17160: output grid (130 rows x 132 cols)
NTILE = 512
NT = -(-NOUT // NTILE)          # 34 spatial tiles
OUT_LEN = NT * NTILE            # 17408
FLAT_SRC = 18944                # padded flat source length (covers max reads)
NB = 4                          # N-tiles per block (weight reuse)
WIN = NB * NTILE + 272          # 2320: rhs window width per block

F32 = mybir.dt.float32
F32R = mybir.dt.float32r
F16 = mybir.dt.float16
ADD = mybir.AluOpType.add
MULT = mybir.AluOpType.mult

# 5 K-chunks over the 9 conv taps p=(kh,kw); flat offset d_p = kh*132+kw.
# Pairs (p_a, p_b): upper/lower SBUF partition halves. Chunks 0-2 pair
# (kh,0)+(kh,1) (delta=1, gtAB buffer), chunk 3 pairs (0,2)+(1,2)
# (delta=132, gtAC buffer), chunk 4 is the lone (2,2) with zeroed lower
# weights.
CHUNKS = [((0, 0), (0, 1)), ((1, 0), (1, 1)), ((2, 0), (2, 1)),
          ((0, 2), (1, 2)), ((2, 2), None)]
# 5 M-tiles: which two j-groups (of the 9 output filter taps) share a PSUM
# tile's upper/lower 64 partitions.
MTILES = CHUNKS


# ------------------------------------------------- TileContext drain patch
# This walrus build rejects >2 sync-wait commands on one CTRL instruction;
# the stock TileContext tail hangs every pending sem wait on a single SP
# Drain. Split them across single-wait SP NOPs (program order on SP still
# places them before the barrier + sem reset).
def _drain_and_barrier_split(self, tick_clock, wait_clock):
    nc = self.nc
    drain_inst = nc.sync.drain()
    wait_clock.add_sem_waits(
        drain_inst.ins, ScopedClock({None: tick_clock.global_clock})
    )
    si = drain_inst.ins.sync_info
    if si is not None and len(si.on_wait) > 1:
        waits = list(si.on_wait)
        drain_inst.ins.sync_info = mybir.SyncInfo(on_wait=[waits[0]], on_update=[])
        for w in waits[1:]:
            nop = nc.sync.nop()
            nop.ins.sync_info = mybir.SyncInfo(on_wait=[w], on_update=[])
    nc.all_engine_barrier()
    assert self.sems is not None
    popped = nc._tile_sem_poison_stack.pop()
    assert popped is self._sem_poison
    nc.clear_and_free_semaphores(list(self.sems.allocated().values()))
    nc.all_engine_barrier()


tile.TileContext._drain_and_barrier = _drain_and_barrier_split


# ------------------------------------------------------------- host prep
def _prep_gt(gt):
    """[C,H,W] -> [C, FLAT_SRC] flat 132x132 grid, 2-ring zero pad."""
    buf = np.zeros((C, FLAT_SRC), np.float32)
    pad = np.zeros((C, NROW, W2), np.float32)
    pad[:, 2:2 + H, 2:2 + W] = gt
    buf[:, :NROW * W2] = pad.reshape(C, -1)
    return buf


def _prep_gr(gr):
    """[C,H,W] -> flat 132x132 grid; inner 130x130 = replicate-padded gr."""
    rp = np.pad(gr, ((0, 0), (1, 1), (1, 1)), mode="edge")
    pad = np.zeros((C, NROW, W2), np.float32)
    pad[:, 1:3 + H, 1:3 + W] = rp
    buf = np.zeros((C, FLAT_SRC), np.float32)
    buf[:, :NROW * W2] = pad.reshape(C, -1)
    return buf


def _jidx(j):
    return j[0] * 3 + j[1]


def _prep_w(Wc):
    """[576,64,3,3] -> [25,128,128] lhsT blocks [(m_tile,chunk), K, M]."""
    out = np.zeros((5, 5, 128, 128), np.float32)
    cc = np.arange(C)
    for m, (j0, j1) in enumerate(MTILES):
        for c, (pa, pb) in enumerate(CHUNKS):
            for hk, p in ((0, pa), (1, pb)):
                if p is None:
                    continue
                kh, kw = p
                for hm, j in ((0, j0), (1, j1)):
                    if j is None:
                        continue
                    blk = Wc[cc * 9 + _jidx(j), :, kh, kw]  # [c_out, i]
                    out[m, c, 64 * hk:64 * hk + 64, 64 * hm:64 * hm + 64] = blk.T
    # partition-major [128, 25*128] so the device load is one plain 2D DMA
    return np.ascontiguousarray(
        out.reshape(25, 128, 128).transpose(1, 0, 2).reshape(128, 25 * 128)
    )


def _prep_b(bc):
    """[576] -> [128,5] per-M-tile per-partition bias (partition-major)."""
    out = np.zeros((5, 128), np.float32)
    cc = np.arange(C)
    for m, (j0, j1) in enumerate(MTILES):
        for hm, j in ((0, j0), (1, j1)):
            if j is None:
                continue
            out[m, 64 * hm:64 * hm + 64] = bc[cc * 9 + _jidx(j)]
    return np.ascontiguousarray(out.T)


# --------------------------------------------------------- bass program
def _build():
    # Bacc (not plain Bass): its finalize() -> compile() legalizes the
    # multi-wait instructions Tile emits (move_matmul_waits_to_ldweights,
    # generate_event_semaphores) which this walrus build otherwise rejects
    # with "Too many sync wait commands".
    nc = bacc.Bacc(None, target_bir_lowering=False)
    gt_src = nc.dram_tensor("gt_src", [C, FLAT_SRC], F32R, kind="ExternalInput")
    gr_src = nc.dram_tensor("gr_src", [C, FLAT_SRC], F32, kind="ExternalInput")
    w_src = nc.dram_tensor("w_src", [128, 25 * 128], F32R, kind="ExternalInput")
    b_src = nc.dram_tensor("b_src", [128, 5], F32, kind="ExternalInput")
    o_dst = nc.dram_tensor("o_dst", [128, OUT_LEN], F16, kind="ExternalOutput")

    sizes = [4] * 8 + [2]
    assert sum(sizes) == NT
    blocks = []
    t0 = 0
    for nb in sizes:
        blocks.append((t0, nb))
        t0 += nb

    # per-M-tile gr source for the product stage: (buffer, flat offset);
    # m3 reads the delta-132 pair buffer full-width, m4 is a half op.
    with tile.TileContext(nc) as tc:
        with (
            tc.tile_pool(name="wpool", bufs=1) as wpool,
            tc.tile_pool(name="winpool", bufs=2) as winpool,
            tc.tile_pool(name="pspool", bufs=4, space="PSUM") as pspool,
            tc.tile_pool(name="prodpool", bufs=12) as prodpool,
            tc.tile_pool(name="accpool", bufs=6) as accpool,
        ):
            # Weights land as partition-major 2D DMAs (a 3-dim AP DMA costs
            # ~11us to issue on SP; per-chunk loads cost ~0.6us each). One
            # DMA per M-tile so the first matmuls only wait for 320KB, not
            # the whole 1.6MB.
            wsb = wpool.tile([128, 25 * 128], F32R, name="wsb", tag="wsb")
            bias_sb = wpool.tile([128, 5], F32, name="bias_sb", tag="bias")

            def load_weights_m(m):
                nc.sync.dma_start(
                    out=wsb[:, m * 640:(m + 1) * 640],
                    in_=w_src[:, m * 640:(m + 1) * 640],
                )

            def stt(out_ap, ps_ap, b_ap, gr_ap):
                nc.vector.scalar_tensor_tensor(
                    out_ap, ps_ap, b_ap, gr_ap, op0=ADD, op1=MULT
                )

            def win_load(pool, name, src, base, pair_step, dtype, wneed):
                """Partitions 0-63 <- src[base+q], 64-127 <-
                src[base+pair_step+q], as two 2D DMAs of just the columns
                this block touches."""
                t = pool.tile([128, WIN], dtype, name=name, tag=name)
                nc.sync.dma_start(out=t[0:64, 0:wneed],
                                  in_=src[:, base:base + wneed])
                nc.sync.dma_start(
                    out=t[64:128, 0:wneed],
                    in_=src[:, base + pair_step:base + pair_step + wneed],
                )
                return t

            for bi, (t0, nb) in enumerate(blocks):
                T = t0 * NTILE
                wneed = nb * NTILE + 272
                gtab = win_load(winpool, "gtab", gt_src, T, 1, F32R, wneed)
                gtac = win_load(winpool, "gtac", gt_src, T + 2, 132, F32R, wneed)
                if bi == 0:
                    # first M-tile's weights right after the first gt windows
                    # so the first matmuls' inputs win the head DMA race;
                    # the rest follow the gr windows
                    load_weights_m(0)
                grab = win_load(winpool, "grab", gr_src, T, 1, F32, wneed)
                grac = win_load(winpool, "grac", gr_src, T + 2, 132, F32, wneed)
                if bi == 0:
                    for m in range(1, 5):
                        load_weights_m(m)
                    nc.sync.dma_start(out=bias_sb[:, :], in_=b_src[:, :])

                # Conv matmuls per M-tile (weights reused across the block's
                # N-tiles to amortize LDWEIGHTS). PSUM tiles span TWO banks
                # (two adjacent N-tiles) so the product/add stage runs
                # 1024-wide DVE ops — halves per-op overhead. Products are
                # written fp16 so the add tree hits the DVE 2x_1P mode.
                npair = nb // 2
                prods = [[None] * 5 for _ in range(npair)]
                for m in range(5):
                    pst = [
                        pspool.tile([128, 2 * NTILE], F32, name=f"ps{m}_{p}",
                                    tag="ps")
                        for p in range(npair)
                    ]
                    for c in range(5):
                        k = m * 5 + c
                        lhsT = wsb[:, k * 128:(k + 1) * 128]
                        for tb in range(nb):
                            q = tb * NTILE
                            if c < 3:
                                rhs = gtab[:, q + c * W2: q + c * W2 + NTILE]
                            elif c == 3:
                                rhs = gtac[:, q: q + NTILE]
                            else:
                                rhs = gtab[:, q + 266: q + 266 + NTILE]
                            out_ps = pst[tb // 2][:, (tb % 2) * NTILE:
                                                  (tb % 2 + 1) * NTILE]
                            nc.tensor.matmul(
                                out_ps, lhsT, rhs,
                                start=(c == 0), stop=(c == 4),
                            )
                    for p in range(npair):
                        q = 2 * p * NTILE
                        Wd = 2 * NTILE
                        pr = prodpool.tile(
                            [128, Wd], F16, name=f"m{m}", tag="prod"
                        )
                        prods[p][m] = pr
                        if m < 3:
                            stt(pr[:, :], pst[p][:, :], bias_sb[:, m:m + 1],
                                grab[:, q + m * W2: q + m * W2 + Wd])
                        elif m == 3:
                            stt(pr[:, :], pst[p][:, :], bias_sb[:, 3:4],
                                grac[:, q: q + Wd])
                        else:
                            stt(pr[0:64], pst[p][0:64], bias_sb[0:64, 4:5],
                                grab[0:64, q + 266: q + 266 + Wd])

                # fp16 pairwise sum tree on DVE (2x_1P mode)
                for p in range(npair):
                    t = t0 + 2 * p
                    Wd = 2 * NTILE
                    m1, m2, m3, m4, m5 = prods[p]
                    a1 = accpool.tile([128, Wd], F16, name="a1", tag="acc")
                    nc.vector.tensor_tensor(a1[:, :], m1[:, :], m2[:, :], op=ADD)
                    a2 = accpool.tile([128, Wd], F16, name="a2", tag="acc")
                    nc.vector.tensor_tensor(a2[:, :], m3[:, :], m4[:, :], op=ADD)
                    a3 = accpool.tile([128, Wd], F16, name="a3", tag="acc")
                    nc.vector.tensor_tensor(a3[:, :], a1[:, :], a2[:, :], op=ADD)
                    nc.vector.tensor_tensor(a3[0:64], a3[0:64], m5[0:64], op=ADD)
                    # out-DMA from the idle ACT queue: on the in-order SP
                    # queue it would park behind the block's last DVE adds
                    # and delay the next block's window loads (~5us PE stall
                    # per block boundary)
                    nc.scalar.dma_start(
                        out=o_dst[:, t * NTILE: t * NTILE + Wd], in_=a3[:, :]
                    )
    nc.finalize()
    return nc


_NC = None


def _get_nc():
    global _NC
    if _NC is None:
        _NC = _build()
    return _NC


_RUN_KW = {}  # test harness can inject trace=True etc.
_LAST_RESULT = None


def kernel(gr, gt, Wc, bc):
    global _LAST_RESULT
    gr = np.ascontiguousarray(np.asarray(gr, dtype=np.float32))
    gt = np.ascontiguousarray(np.asarray(gt, dtype=np.float32))
    Wc = np.asarray(Wc, dtype=np.float32)
    bc = np.asarray(bc, dtype=np.float32)

    wb = _prep_w(Wc)
    bb = _prep_b(bc)
    in_maps = [
        {
            "gt_src": _prep_gt(gt[n]),
            "gr_src": _prep_gr(gr[n]),
            "w_src": wb,
            "b_src": bb,
        }
        for n in range(N)
    ]
    res = run_bass_kernel_spmd(
        _get_nc(), in_maps, core_ids=list(range(N)), **_RUN_KW
    )
    _LAST_RESULT = res

    hh = np.arange(H)
    cols = ((hh + 1) * W2)[:, None] + (np.arange(W) + 1)[None, :]
    outs = []
    for n in range(N):
        O = res.results[n]["o_dst"].astype(np.float32)
        flat = O[:64] + O[64:]
        outs.append(flat[:, cols])
    return np.stack(outs).astype(np.float32)

